# revision 16
# baseline (speedup 1.0000x reference)
"""Trainium2 Bass kernel for the 2-layer LSTM bar decoder.

Model (per bar, 16 bars, all sharing weights):
  16 steps of: x = [out, emb]; (h0,c0)=LSTMCell0(x); (h1,c1)=LSTMCell1(h0);
  out = softmax(h1 @ W_out.T + b_out)

Strategy:
  - Data-parallel over (bar, batch): 16 bars x 256 batch = 4096 independent
    rows; each of 8 cores owns a 32-batch slice x all bars = 512 rows.
  - All state kept TRANSPOSED in SBUF ([hidden, rows], hidden on partitions)
    so every matmul uses the small weights as the stationary operand and the
    512-row state as the moving operand (N=512, full PE efficiency).
  - The four big recurrent matmuls (W_hh0/W_ih0a/W_hh1/W_ih1) run in
    fp8e4 with DoubleRow (K=256 per instruction, ~1.8x PE throughput).
    Weights and fp8 states carry a x32 scale (PSUM = 1024 x true value),
    compensated exactly in the ACT input scales. fp32 PSUM accumulation;
    cell state, gates, logits and softmax stay fp16/fp32.
  - emb contribution of layer-0 gates is step-invariant: computed once
    (fp16, x1024), injected per step as an identity-matmul accumulation
    (keeps DVE off the critical path).
  - sigmoid built from tanh (sigma(x) = 0.5 + 0.5*tanh(x/2)) so the whole
    kernel needs a single ACT table set (exp_and_others: tanh + exp).
  - softmax in ROW-MAJOR domain: logits via matmul with the h1 slice as
    stationary and W_out^T as moving (same PE cost as transposed layout),
    b_out folded in via a K=1 ones-matmul, exp with fused free-axis
    accumulation (softmax denominator for free), tiny per-partition
    reciprocals, ACT per-partition-scale normalization. The fp32 output
    tile DMAs straight out (row-major IS the output layout); only the
    next-step recurrence input is transposed back on the PE.
"""

import os
import sys

import numpy as np

H = 512
BARS = 16
UNITS = 16
B = 256
NCORES = 8
BPC = B // NCORES  # batch rows per core
R = BARS * BPC  # rows per core (bar-major)
GT = (4 * H) // 128  # gate tiles per layer
KT = H // 128  # k (hidden) tiles
RT = R // 128  # row tiles
FS = 32.0  # fp8 operand scale (weights and moving states)
FS2 = FS * FS  # PSUM scale of the fp8 gate matmuls

LAST_EXEC_NS = None

_cache = {}


def _ensure_path():
    for p in ("/opt/trn_rl_repo",):
        if os.path.isdir(p) and p not in sys.path:
            sys.path.insert(0, p)


def _build_nc():
    _ensure_path()
    import concourse.tile as tile
    from concourse import bacc, mybir
    from concourse.masks import make_identity

    f16 = mybir.dt.float16
    f32 = mybir.dt.float32
    f8 = mybir.dt.float8e4
    AF = mybir.ActivationFunctionType
    ALU = mybir.AluOpType
    DR = mybir.MatmulPerfMode.DoubleRow

    nc = bacc.Bacc("TRN2")

    w0a = nc.declare_dram_parameter("w0a", [128, KT, 4 * H], f8, isOutput=False)
    w0b = nc.declare_dram_parameter("w0b", [H, 4 * H], f16, isOutput=False)
    w0h = nc.declare_dram_parameter("w0h", [128, KT, 4 * H], f8, isOutput=False)
    w1i = nc.declare_dram_parameter("w1i", [128, KT, 4 * H], f8, isOutput=False)
    w1h = nc.declare_dram_parameter("w1h", [128, KT, 4 * H], f8, isOutput=False)
    wo = nc.declare_dram_parameter("wo", [H, H], f16, isOutput=False)
    b0 = nc.declare_dram_parameter("b0", [128, GT], f32, isOutput=False)
    b1h = nc.declare_dram_parameter("b1h", [128, GT], f32, isOutput=False)
    b1f = nc.declare_dram_parameter("b1f", [128, GT], f32, isOutput=False)
    bo_row = nc.declare_dram_parameter("bo_row", [1, H], f16, isOutput=False)
    embT = nc.declare_dram_parameter("embT", [H, R], f16, isOutput=False)
    h0T8 = nc.declare_dram_parameter("h0T8", [128, KT, R], f8, isOutput=False)
    h1T8 = nc.declare_dram_parameter("h1T8", [128, KT, R], f8, isOutput=False)
    c0T = nc.declare_dram_parameter("c0T", [H, R], f16, isOutput=False)
    c1T = nc.declare_dram_parameter("c1T", [H, R], f16, isOutput=False)
    oT8 = nc.declare_dram_parameter("oT8", [128, KT, R], f8, isOutput=False)
    out = nc.declare_dram_parameter("out", [BPC, BARS * UNITS, H], f32, isOutput=True)

    # [b, bar*16+u, h] viewed as [bar, u, b, h] for per-(step,row-tile) stores
    out_v = out[:, :, :].rearrange("b (bar u) h -> bar u b h", bar=BARS)

    with tile.TileContext(nc) as tc:
        with (
            tc.tile_pool(name="consts", bufs=1) as consts,
            tc.tile_pool(name="wpool", bufs=1) as wpool,
            tc.tile_pool(name="eppool", bufs=1) as eppool,
            tc.tile_pool(name="cpool", bufs=1) as cpool,
            tc.tile_pool(name="hpool", bufs=2) as hpool,
            tc.tile_pool(name="gsb", bufs=1) as gsb,
            tc.tile_pool(name="cellsb", bufs=2) as cellsb,
            tc.tile_pool(name="smx", bufs=2) as smx,
            tc.tile_pool(name="pg", bufs=6, space="PSUM") as pg,
            tc.tile_pool(name="plog", bufs=2, space="PSUM") as plog,
        ):
            ident = consts.tile([128, 128], f16, tag="ident")
            make_identity(nc, ident)
            ones16 = consts.tile([1, 128], f16, tag="ones16")
            nc.vector.memset(ones16, 1.0)

            # warmup ops with minimal sync waits so the implicit ACT/DVE
            # table loads don't attach to instructions that already carry
            # multiple semaphore waits (walrus sync-wait limit).
            warm = consts.tile([128, 1], f32, tag="warm")
            nc.vector.memset(warm, 1.0)
            nc.scalar.activation(warm[:, :], warm[:, :], AF.Tanh)
            nc.scalar.activation(warm[:, :], warm[:, :], AF.Exp)
            nc.vector.reciprocal(warm[:, :], warm[:, :])

            b0_sb = consts.tile([128, GT], f32, tag="b0")
            nc.sync.dma_start(out=b0_sb, in_=b0[:, :])
            b1h_sb = consts.tile([128, GT], f32, tag="b1h")
            nc.sync.dma_start(out=b1h_sb, in_=b1h[:, :])
            b1f_sb = consts.tile([128, GT], f32, tag="b1f")
            nc.sync.dma_start(out=b1f_sb, in_=b1f[:, :])
            bo_sb = consts.tile([1, H], f16, tag="bo_row")
            nc.sync.dma_start(out=bo_sb, in_=bo_row[:, :])

            def load_w8(dram, name):
                t = wpool.tile([128, KT, 4 * H], f8, tag=name)
                nc.sync.dma_start(out=t[:, :, :], in_=dram[:, :, :])
                return t

            def load_s8(dram, pool, name):
                t = pool.tile([128, KT, R], f8, tag=name)
                nc.sync.dma_start(out=t[:, :, :], in_=dram[:, :, :])
                return t

            def load_ktiles(dram, pool, name, free, dtype):
                ts = []
                for k in range(KT):
                    t = pool.tile([128, free], dtype, tag=f"{name}{k}")
                    nc.sync.dma_start(out=t[:, :], in_=dram[k * 128 : (k + 1) * 128, :])
                    ts.append(t)
                return ts

            w0a_sb = load_w8(w0a, "w0a")
            w0h_sb = load_w8(w0h, "w0h")
            w1i_sb = load_w8(w1i, "w1i")
            w1h_sb = load_w8(w1h, "w1h")
            wo_sb = load_ktiles(wo, wpool, "wo", H, f16)

            cur_h0 = load_s8(h0T8, hpool, "h0_8")
            cur_h1 = load_s8(h1T8, hpool, "h1_8")
            cur_o = load_s8(oT8, hpool, "o_8")
            c0_sb = load_ktiles(c0T, cpool, "c0_", R, f16)
            c1_sb = load_ktiles(c1T, cpool, "c1_", R, f16)

            # -------- precompute: embpre[gt] = FS2*(W_ih0_emb @ embT + b0)[gt]
            # (fp16 matmul; b0 pre-scaled by FS2 on the host)
            embpre = []
            with tc.tile_pool(name="prepool", bufs=1) as prepool:
                embT_sb = load_ktiles(embT, prepool, "embT", R, f16)
                w0b_sb = load_ktiles(w0b, prepool, "w0b", 4 * H, f16)
                for gt in range(GT):
                    ps = pg.tile([128, R], f32, tag="g")
                    for k in range(KT):
                        nc.tensor.matmul(
                            ps[:, :],
                            w0b_sb[k][:, gt * 128 : (gt + 1) * 128],
                            embT_sb[k][:, :],
                            start=(k == 0),
                            stop=(k == KT - 1),
                        )
                    ep = eppool.tile([128, R], f16, tag=f"ep{gt}")
                    nc.scalar.activation(
                        ep[:, :],
                        ps[:, :],
                        AF.Identity,
                        bias=b0_sb[:, gt : gt + 1],
                        scale=FS2,
                    )
                    embpre.append(ep)

            # -------- recurrence --------
            def lstm_layer(w_h, h8, w_x, x8, emb_add, bias_half, bias_full, c_sb, mk_nh):
                """One LSTM layer, fp8 DoubleRow gates. PSUM = FS2 * true gates
                (+ emb_add which is already FS2-scaled). mk_nh(ht) returns the
                destination AP for the new h k-tile; returns list of those.

                Gates are emitted in [ht, 4+ht, 8+ht, 12+ht] order with the
                ht cell update nested right after, so each new-h k-tile is
                ready 4x earlier and the next consumer never stalls the PE."""

                def gate(gt):
                    ps = pg.tile([128, R], f32, tag="g")
                    if emb_add is not None:
                        nc.tensor.matmul(
                            ps[:, :],
                            ident[:, :],
                            emb_add[gt][:, :],
                            start=True,
                            stop=False,
                        )
                    for ks in range(0, KT, 2):
                        nc.tensor.matmul(
                            ps[:, :],
                            w_h[:, ks : ks + 2, gt * 128 : (gt + 1) * 128],
                            h8[:, ks : ks + 2, :],
                            start=(emb_add is None and ks == 0),
                            stop=False,
                            perf_mode=DR,
                        )
                    for ks in range(0, KT, 2):
                        nc.tensor.matmul(
                            ps[:, :],
                            w_x[:, ks : ks + 2, gt * 128 : (gt + 1) * 128],
                            x8[:, ks : ks + 2, :],
                            start=False,
                            stop=(ks == KT - 2),
                            perf_mode=DR,
                        )
                    tgt = gsb.tile([128, R], f16, tag=f"tg{gt}")
                    if 8 <= gt < 12:  # g gate: tanh(x)
                        bias = bias_full
                        sc = 1.0 / FS2
                    else:  # i/f/o gates: tanh(x/2) -> sigmoid
                        bias = bias_half
                        sc = 0.5 / FS2
                    if bias is not None:
                        nc.scalar.activation(
                            tgt[:, :], ps[:, :], AF.Tanh,
                            bias=bias[:, gt : gt + 1], scale=sc,
                        )
                    else:
                        nc.scalar.activation(tgt[:, :], ps[:, :], AF.Tanh, scale=sc)
                    return tgt

                outs = [None] * KT
                for ht in range(KT):
                    ti = gate(ht)
                    tf = gate(4 + ht)
                    tgg = gate(8 + ht)
                    to = gate(12 + ht)
                    # sigma = 0.5*tanh + 0.5 (o gate carries the FS scale)
                    nc.vector.tensor_scalar(ti[:, :], ti[:, :], 0.5, 0.5, ALU.mult, ALU.add)
                    nc.vector.tensor_scalar(tf[:, :], tf[:, :], 0.5, 0.5, ALU.mult, ALU.add)
                    nc.vector.tensor_scalar(
                        to[:, :], to[:, :], FS / 2, FS / 2, ALU.mult, ALU.add
                    )
                    m1 = cellsb.tile([128, R], f16, tag=f"m1_{ht}")
                    nc.vector.tensor_mul(m1[:, :], tf[:, :], c_sb[ht][:, :])
                    nc.vector.tensor_mul(ti[:, :], ti[:, :], tgg[:, :])
                    nc.vector.tensor_add(c_sb[ht][:, :], m1[:, :], ti[:, :])
                    tch = cellsb.tile([128, R], f16, tag=f"tc_{ht}")
                    nc.scalar.activation(tch[:, :], c_sb[ht][:, :], AF.Tanh)
                    nh = mk_nh(ht)
                    nc.vector.tensor_mul(nh, to[:, :], tch[:, :])
                    outs[ht] = nh
                return outs

            repeat = int(os.environ.get("KREPEAT", "1"))
            for t in list(range(UNITS)) * repeat:
                nh0_8 = hpool.tile([128, KT, R], f8, tag="h0_8")
                lstm_layer(
                    w0h_sb, cur_h0, w0a_sb, cur_o, embpre, None, None, c0_sb,
                    lambda ht: nh0_8[:, ht, :],
                )
                # layer 1 h is needed in fp16 (x FS) as the logits stationary
                nh1_16 = []
                for ht in range(KT):
                    h1f = gsb.tile([128, R], f16, tag=f"h1f{ht}")
                    nh1_16.append(h1f)
                lstm_layer(
                    w1h_sb, cur_h1, w1i_sb, nh0_8, None, b1h_sb, b1f_sb, c1_sb,
                    lambda ht: nh1_16[ht][:, :],
                )
                nh1_8 = hpool.tile([128, KT, R], f8, tag="h1_8")
                for k in range(KT):
                    nc.gpsimd.tensor_copy(nh1_8[:, k, :], nh1_16[k][:, :])

                # row-major logits: psum[row, m] = FS*(h1 @ WoT + b_out)
                o16 = [None] * RT
                for rt in range(RT):
                    ps = plog.tile([128, H], f32, tag="lg")
                    for k in range(KT):
                        nc.tensor.matmul(
                            ps[:, :],
                            nh1_16[k][:, rt * 128 : (rt + 1) * 128],
                            wo_sb[k][:, :],
                            start=(k == 0),
                            stop=False,
                        )
                    nc.tensor.matmul(
                        ps[:, :], ones16[:, :], bo_sb[:, :], start=False, stop=True
                    )
                    e = smx.tile([128, H], f16, tag=f"e{rt}")
                    ssum = smx.tile([128, 1], f32, tag=f"s{rt}")
                    nc.scalar.activation(
                        e[:, :], ps[:, :], AF.Exp, scale=1.0 / FS,
                        accum_out=ssum[:, :],
                    )
                    rec = smx.tile([128, 1], f32, tag=f"r{rt}")
                    nc.vector.reciprocal(rec[:, :], ssum[:, :])
                    o = gsb.tile([128, H], f16, tag=f"o16_{rt}")
                    nc.vector.tensor_scalar(o[:, :], e[:, :], rec[:, :], None, ALU.mult)
                    o16[rt] = o
                    o32 = smx.tile([128, H], f32, tag=f"o32_{rt}")
                    nc.vector.tensor_scalar(
                        o32[:, :], e[:, :], rec[:, :], None, ALU.mult
                    )
                    nc.sync.dma_start(
                        out=out_v[rt * 4 : (rt + 1) * 4, t, :, :], in_=o32[:, :]
                    )

                # transpose normalized out back to [hidden, rows] (x FS, fp8)
                # for the next step's layer-0 x input — on the DMA xbar
                # transpose engines (PE stays free for matmuls)
                no_8 = hpool.tile([128, KT, R], f8, tag="o_8")
                for hc in range(KT):
                    ot16 = cellsb.tile([128, R], f16, tag=f"oT16_{hc}")
                    for rt in range(RT):
                        nc.sync.dma_start_transpose(
                            out=ot16[:, rt * 128 : (rt + 1) * 128],
                            in_=o16[rt][:, hc * 128 : (hc + 1) * 128],
                        )
                    nc.vector.tensor_scalar(
                        no_8[:, hc, :], ot16[:, :], FS, None, ALU.mult
                    )

                cur_h0, cur_h1, cur_o = nh0_8, nh1_8, no_8

    return nc


def _get_nc():
    if "nc" not in _cache:
        nc = _build_nc()
        if not nc.is_finalized():
            nc.finalize()
        _cache["nc"] = nc
    return _cache["nc"]


def _f8(x):
    import ml_dtypes

    return np.clip(x, -240.0, 240.0).astype(ml_dtypes.float8_e4m3)


def _pack8(mat, scale):
    """[H, F] (k-major) -> [128, KT, F] fp8, scaled."""
    h, f = mat.shape
    assert h == H
    return _f8(np.ascontiguousarray(mat.reshape(KT, 128, f).transpose(1, 0, 2)) * scale)


def _make_in_maps(inputs):
    x = {k: np.asarray(v) for k, v in inputs.items()}
    W_ih0 = x["W_ih0"].astype(np.float32)
    W_hh0 = x["W_hh0"].astype(np.float32)
    W_ih1 = x["W_ih1"].astype(np.float32)
    W_hh1 = x["W_hh1"].astype(np.float32)
    W_out = x["W_out"].astype(np.float32)
    b0 = (x["b_ih0"] + x["b_hh0"]).astype(np.float32)
    b1 = (x["b_ih1"] + x["b_hh1"]).astype(np.float32)
    b_out = x["b_out"].astype(np.float32)
    emb = x["embedding_C"].astype(np.float32)
    h0 = x["h0"].astype(np.float32)
    c0 = x["c0"].astype(np.float32)
    out0 = x["out0"].astype(np.float32)

    FS2_np = np.float32(FS * FS)
    shared = {
        "w0a": _pack8(np.ascontiguousarray(W_ih0[:, :H].T), FS),
        "w0b": np.ascontiguousarray(W_ih0[:, H:].T).astype(np.float16),
        "w0h": _pack8(np.ascontiguousarray(W_hh0.T), FS),
        "w1i": _pack8(np.ascontiguousarray(W_ih1.T), FS),
        "w1h": _pack8(np.ascontiguousarray(W_hh1.T), FS),
        "wo": np.ascontiguousarray(W_out.T).astype(np.float16),
        "b0": np.ascontiguousarray((b0 * FS2_np).reshape(GT, 128).T),
        "b1h": np.ascontiguousarray((b1 * 0.5).reshape(GT, 128).T),
        "b1f": np.ascontiguousarray(b1.reshape(GT, 128).T),
        "bo_row": np.ascontiguousarray((b_out * FS).reshape(1, H)).astype(np.float16),
    }

    def t16(rows_by_h):  # [R, H] -> [H, R] fp16
        return np.ascontiguousarray(rows_by_h.T).astype(np.float16)

    in_maps = []
    for c in range(NCORES):
        bs = slice(c * BPC, (c + 1) * BPC)
        m = dict(shared)
        m["embT"] = t16(np.swapaxes(emb[bs], 0, 1).reshape(R, H))
        m["h0T8"] = _pack8(h0[:, 0, bs, :].reshape(R, H).T, FS)
        m["h1T8"] = _pack8(h0[:, 1, bs, :].reshape(R, H).T, FS)
        m["c0T"] = t16(c0[:, 0, bs, :].reshape(R, H))
        m["c1T"] = t16(c0[:, 1, bs, :].reshape(R, H))
        m["oT8"] = _pack8(out0[:, bs, :].reshape(R, H).T, FS)
        in_maps.append(m)
    return in_maps


def kernel(**inputs):
    global LAST_EXEC_NS
    _ensure_path()
    from concourse.bass_utils import run_bass_kernel_spmd

    in_maps = _make_in_maps(inputs)
    nc = _get_nc()
    trace = bool(os.environ.get("KTRACE"))
    kw = {}
    if trace and os.environ.get("KTRACE_DIR"):
        os.makedirs(os.environ["KTRACE_DIR"], exist_ok=True)
        kw["tmpdir"] = os.environ["KTRACE_DIR"]
    try:
        res = run_bass_kernel_spmd(nc, in_maps, list(range(NCORES)), trace=trace, **kw)
    except (ImportError, ModuleNotFoundError):
        res = run_bass_kernel_spmd(nc, in_maps, list(range(NCORES)), trace=False)
    if getattr(res, "exec_time_ns", None):
        LAST_EXEC_NS = res.exec_time_ns

    outs = [np.asarray(res.results[c]["out"], dtype=np.float32) for c in range(NCORES)]
    return np.concatenate(outs, axis=0)


if __name__ == "__main__":
    nc = _get_nc()
    print("built ok")


# revision 19
# speedup vs baseline: 1.2737x; 1.2737x over previous
"""Trainium2 Bass kernel for the 2-layer LSTM bar decoder.

Model (per bar, 16 bars, all sharing weights):
  16 steps of: x = [out, emb]; (h0,c0)=LSTMCell0(x); (h1,c1)=LSTMCell1(h0);
  out = softmax(h1 @ W_out.T + b_out)

Strategy:
  - Data-parallel over (bar, batch): 16 bars x 256 batch = 4096 independent
    rows; each of 8 cores owns a 32-batch slice x all bars = 512 rows.
  - All state kept TRANSPOSED in SBUF ([hidden, rows], hidden on partitions)
    so every matmul uses the small weights as the stationary operand and the
    512-row state as the moving operand (N=512, full PE efficiency).
  - The four big recurrent matmuls (W_hh0/W_ih0a/W_hh1/W_ih1) run in
    fp8e4 with DoubleRow (K=256 per instruction, ~1.8x PE throughput).
    Weights and fp8 states carry a x32 scale (PSUM = 1024 x true value),
    compensated exactly in the ACT input scales. fp32 PSUM accumulation;
    cell state, gates, logits and softmax stay fp16/fp32.
  - emb contribution of layer-0 gates is step-invariant: computed once
    (fp16, x1024), injected per step as an identity-matmul accumulation
    (keeps DVE off the critical path).
  - sigmoid built from tanh (sigma(x) = 0.5 + 0.5*tanh(x/2)) so the whole
    kernel needs a single ACT table set (exp_and_others: tanh + exp).
  - softmax in ROW-MAJOR domain: logits via matmul with the h1 slice as
    stationary and W_out^T as moving (same PE cost as transposed layout),
    b_out folded in via a K=1 ones-matmul, exp with fused free-axis
    accumulation (softmax denominator for free), tiny per-partition
    reciprocals, ACT per-partition-scale normalization. The fp32 output
    tile DMAs straight out (row-major IS the output layout); only the
    next-step recurrence input is transposed back on the PE.
"""

import os
import sys

import numpy as np

H = 512
BARS = 16
UNITS = 16
B = 256
NCORES = 8
BPC = B // NCORES  # batch rows per core
R = BARS * BPC  # rows per core (bar-major)
GT = (4 * H) // 128  # gate tiles per layer
KT = H // 128  # k (hidden) tiles
RT = R // 128  # row tiles
FS = 32.0  # fp8 operand scale (weights and moving states)
FS2 = FS * FS  # PSUM scale of the fp8 gate matmuls

LAST_EXEC_NS = None

_cache = {}


def _ensure_path():
    for p in ("/opt/trn_rl_repo",):
        if os.path.isdir(p) and p not in sys.path:
            sys.path.insert(0, p)


def _build_nc():
    _ensure_path()
    import concourse.tile as tile
    from concourse import bacc, mybir
    from concourse.masks import make_identity

    f16 = mybir.dt.float16
    f32 = mybir.dt.float32
    f8 = mybir.dt.float8e4
    AF = mybir.ActivationFunctionType
    ALU = mybir.AluOpType
    DR = mybir.MatmulPerfMode.DoubleRow

    nc = bacc.Bacc("TRN2")

    w0a = nc.declare_dram_parameter("w0a", [128, KT, 4 * H], f8, isOutput=False)
    w0b = nc.declare_dram_parameter("w0b", [H, 4 * H], f16, isOutput=False)
    w0h = nc.declare_dram_parameter("w0h", [128, KT, 4 * H], f8, isOutput=False)
    w1i = nc.declare_dram_parameter("w1i", [128, KT, 4 * H], f8, isOutput=False)
    w1h = nc.declare_dram_parameter("w1h", [128, KT, 4 * H], f8, isOutput=False)
    wo = nc.declare_dram_parameter("wo", [H, H], f16, isOutput=False)
    b0 = nc.declare_dram_parameter("b0", [128, GT], f32, isOutput=False)
    b1h = nc.declare_dram_parameter("b1h", [128, GT], f32, isOutput=False)
    b1f = nc.declare_dram_parameter("b1f", [128, GT], f32, isOutput=False)
    bo_row = nc.declare_dram_parameter("bo_row", [1, H], f16, isOutput=False)
    embT = nc.declare_dram_parameter("embT", [H, R], f16, isOutput=False)
    h0T8 = nc.declare_dram_parameter("h0T8", [128, KT, R], f8, isOutput=False)
    h1T8 = nc.declare_dram_parameter("h1T8", [128, KT, R], f8, isOutput=False)
    c0T = nc.declare_dram_parameter("c0T", [H, R], f16, isOutput=False)
    c1T = nc.declare_dram_parameter("c1T", [H, R], f16, isOutput=False)
    oT8 = nc.declare_dram_parameter("oT8", [128, KT, R], f8, isOutput=False)
    out = nc.declare_dram_parameter("out", [BPC, BARS * UNITS, H], f32, isOutput=True)

    # [b, bar*16+u, h] viewed as [bar, u, b, h] for per-(step,row-tile) stores
    out_v = out[:, :, :].rearrange("b (bar u) h -> bar u b h", bar=BARS)

    with tile.TileContext(nc) as tc:
        with (
            tc.tile_pool(name="consts", bufs=1) as consts,
            tc.tile_pool(name="wpool", bufs=1) as wpool,
            tc.tile_pool(name="eppool", bufs=1) as eppool,
            tc.tile_pool(name="cpool", bufs=1) as cpool,
            tc.tile_pool(name="hpool", bufs=2) as hpool,
            tc.tile_pool(name="gsb", bufs=1) as gsb,
            tc.tile_pool(name="cellsb", bufs=2) as cellsb,
            tc.tile_pool(name="smx", bufs=2) as smx,
            tc.tile_pool(name="pg", bufs=4, space="PSUM") as pg,
            tc.tile_pool(name="plog", bufs=2, space="PSUM") as plog,
            tc.tile_pool(name="poT", bufs=2, space="PSUM") as poT,
        ):
            ident = consts.tile([128, 128], f16, tag="ident")
            make_identity(nc, ident)
            ones16 = consts.tile([1, 128], f16, tag="ones16")
            nc.vector.memset(ones16, 1.0)

            # warmup ops with minimal sync waits so the implicit ACT/DVE
            # table loads don't attach to instructions that already carry
            # multiple semaphore waits (walrus sync-wait limit).
            warm = consts.tile([128, 1], f32, tag="warm")
            nc.vector.memset(warm, 1.0)
            nc.scalar.activation(warm[:, :], warm[:, :], AF.Tanh)
            nc.scalar.activation(warm[:, :], warm[:, :], AF.Exp)
            nc.vector.reciprocal(warm[:, :], warm[:, :])

            b0_sb = consts.tile([128, GT], f32, tag="b0")
            nc.sync.dma_start(out=b0_sb, in_=b0[:, :])
            b1h_sb = consts.tile([128, GT], f32, tag="b1h")
            nc.sync.dma_start(out=b1h_sb, in_=b1h[:, :])
            b1f_sb = consts.tile([128, GT], f32, tag="b1f")
            nc.sync.dma_start(out=b1f_sb, in_=b1f[:, :])
            bo_sb = consts.tile([1, H], f16, tag="bo_row")
            nc.sync.dma_start(out=bo_sb, in_=bo_row[:, :])

            def load_w8(dram, name):
                t = wpool.tile([128, KT, 4 * H], f8, tag=name)
                nc.sync.dma_start(out=t[:, :, :], in_=dram[:, :, :])
                return t

            def load_s8(dram, pool, name):
                t = pool.tile([128, KT, R], f8, tag=name)
                nc.sync.dma_start(out=t[:, :, :], in_=dram[:, :, :])
                return t

            def load_ktiles(dram, pool, name, free, dtype):
                ts = []
                for k in range(KT):
                    t = pool.tile([128, free], dtype, tag=f"{name}{k}")
                    nc.sync.dma_start(out=t[:, :], in_=dram[k * 128 : (k + 1) * 128, :])
                    ts.append(t)
                return ts

            w0a_sb = load_w8(w0a, "w0a")
            w0h_sb = load_w8(w0h, "w0h")
            w1i_sb = load_w8(w1i, "w1i")
            w1h_sb = load_w8(w1h, "w1h")
            wo_sb = load_ktiles(wo, wpool, "wo", H, f16)

            cur_h0 = load_s8(h0T8, hpool, "h0_8")
            cur_h1 = load_s8(h1T8, hpool, "h1_8")
            cur_o = load_s8(oT8, hpool, "o_8")
            c0_sb = load_ktiles(c0T, cpool, "c0_", R, f16)
            c1_sb = load_ktiles(c1T, cpool, "c1_", R, f16)

            # -------- precompute: embpre[gt] = FS2*(W_ih0_emb @ embT + b0)[gt]
            # (fp16 matmul; b0 pre-scaled by FS2 on the host)
            embpre = []
            with tc.tile_pool(name="prepool", bufs=1) as prepool:
                embT_sb = load_ktiles(embT, prepool, "embT", R, f16)
                w0b_sb = load_ktiles(w0b, prepool, "w0b", 4 * H, f16)
                for gt in range(GT):
                    ps = pg.tile([128, R], f32, tag="g")
                    for k in range(KT):
                        nc.tensor.matmul(
                            ps[:, :],
                            w0b_sb[k][:, gt * 128 : (gt + 1) * 128],
                            embT_sb[k][:, :],
                            start=(k == 0),
                            stop=(k == KT - 1),
                        )
                    ep = eppool.tile([128, R], f16, tag=f"ep{gt}")
                    nc.scalar.activation(
                        ep[:, :],
                        ps[:, :],
                        AF.Identity,
                        bias=b0_sb[:, gt : gt + 1],
                        scale=FS2,
                    )
                    embpre.append(ep)

            # -------- recurrence --------
            def lstm_layer(w_h, h8, w_x, x8, emb_add, bias_half, bias_full, c_sb, mk_nh):
                """One LSTM layer, fp8 DoubleRow gates. PSUM = FS2 * true gates
                (+ emb_add which is already FS2-scaled). mk_nh(ht) returns the
                destination AP for the new h k-tile; returns list of those.

                Gates are emitted in [ht, 4+ht, 8+ht, 12+ht] order with the
                ht cell update nested right after, so each new-h k-tile is
                ready 4x earlier and the next consumer never stalls the PE."""

                def gate(gt):
                    ps = pg.tile([128, R], f32, tag="g")
                    if emb_add is not None:
                        nc.tensor.matmul(
                            ps[:, :],
                            ident[:, :],
                            emb_add[gt][:, :],
                            start=True,
                            stop=False,
                        )
                    for ks in range(0, KT, 2):
                        nc.tensor.matmul(
                            ps[:, :],
                            w_h[:, ks : ks + 2, gt * 128 : (gt + 1) * 128],
                            h8[:, ks : ks + 2, :],
                            start=(emb_add is None and ks == 0),
                            stop=False,
                            perf_mode=DR,
                        )
                    for ks in range(0, KT, 2):
                        nc.tensor.matmul(
                            ps[:, :],
                            w_x[:, ks : ks + 2, gt * 128 : (gt + 1) * 128],
                            x8[:, ks : ks + 2, :],
                            start=False,
                            stop=(ks == KT - 2),
                            perf_mode=DR,
                        )
                    tgt = gsb.tile([128, R], f16, tag=f"tg{gt}")
                    if 8 <= gt < 12:  # g gate: tanh(x)
                        bias = bias_full
                        sc = 1.0 / FS2
                    else:  # i/f/o gates: tanh(x/2) -> sigmoid
                        bias = bias_half
                        sc = 0.5 / FS2
                    if bias is not None:
                        nc.scalar.activation(
                            tgt[:, :], ps[:, :], AF.Tanh,
                            bias=bias[:, gt : gt + 1], scale=sc,
                        )
                    else:
                        nc.scalar.activation(tgt[:, :], ps[:, :], AF.Tanh, scale=sc)
                    return tgt

                outs = [None] * KT
                for ht in range(KT):
                    ti = gate(ht)
                    tf = gate(4 + ht)
                    tgg = gate(8 + ht)
                    to = gate(12 + ht)
                    # sigma = 0.5*tanh + 0.5 (o gate carries the FS scale)
                    nc.vector.tensor_scalar(ti[:, :], ti[:, :], 0.5, 0.5, ALU.mult, ALU.add)
                    nc.vector.tensor_scalar(tf[:, :], tf[:, :], 0.5, 0.5, ALU.mult, ALU.add)
                    nc.vector.tensor_scalar(
                        to[:, :], to[:, :], FS / 2, FS / 2, ALU.mult, ALU.add
                    )
                    m1 = cellsb.tile([128, R], f16, tag=f"m1_{ht}")
                    nc.vector.tensor_mul(m1[:, :], tf[:, :], c_sb[ht][:, :])
                    nc.vector.tensor_mul(ti[:, :], ti[:, :], tgg[:, :])
                    nc.vector.tensor_add(c_sb[ht][:, :], m1[:, :], ti[:, :])
                    tch = cellsb.tile([128, R], f16, tag=f"tc_{ht}")
                    nc.scalar.activation(tch[:, :], c_sb[ht][:, :], AF.Tanh)
                    nh = mk_nh(ht)
                    nc.vector.tensor_mul(nh, to[:, :], tch[:, :])
                    outs[ht] = nh
                return outs

            repeat = int(os.environ.get("KREPEAT", "1"))
            for t in list(range(UNITS)) * repeat:
                nh0_8 = hpool.tile([128, KT, R], f8, tag="h0_8")
                lstm_layer(
                    w0h_sb, cur_h0, w0a_sb, cur_o, embpre, None, None, c0_sb,
                    lambda ht: nh0_8[:, ht, :],
                )
                # layer 1 h is needed in fp16 (x FS) as the logits stationary
                nh1_16 = []
                for ht in range(KT):
                    h1f = gsb.tile([128, R], f16, tag=f"h1f{ht}")
                    nh1_16.append(h1f)
                lstm_layer(
                    w1h_sb, cur_h1, w1i_sb, nh0_8, None, b1h_sb, b1f_sb, c1_sb,
                    lambda ht: nh1_16[ht][:, :],
                )
                nh1_8 = hpool.tile([128, KT, R], f8, tag="h1_8")
                for k in range(KT):
                    nc.gpsimd.tensor_copy(nh1_8[:, k, :], nh1_16[k][:, :])

                # row-major logits: psum[row, m] = FS*(h1 @ WoT + b_out)
                o16 = [None] * RT
                for rt in range(RT):
                    ps = plog.tile([128, H], f32, tag="lg")
                    for k in range(KT):
                        nc.tensor.matmul(
                            ps[:, :],
                            nh1_16[k][:, rt * 128 : (rt + 1) * 128],
                            wo_sb[k][:, :],
                            start=(k == 0),
                            stop=False,
                        )
                    nc.tensor.matmul(
                        ps[:, :], ones16[:, :], bo_sb[:, :], start=False, stop=True
                    )
                    e = smx.tile([128, H], f16, tag=f"e{rt}")
                    ssum = smx.tile([128, 1], f32, tag=f"s{rt}")
                    nc.scalar.activation(
                        e[:, :], ps[:, :], AF.Exp, scale=1.0 / FS,
                        accum_out=ssum[:, :],
                    )
                    rec = smx.tile([128, 1], f32, tag=f"r{rt}")
                    nc.vector.reciprocal(rec[:, :], ssum[:, :])
                    o = gsb.tile([128, H], f16, tag=f"o16_{rt}")
                    nc.vector.tensor_scalar(o[:, :], e[:, :], rec[:, :], None, ALU.mult)
                    o16[rt] = o

                # transpose normalized out back to [hidden, rows] (x FS, fp8)
                # for the next step's layer-0 x input
                no_8 = hpool.tile([128, KT, R], f8, tag="o_8")
                for hc in range(KT):
                    pt = poT.tile([128, R], f16, tag="oT")
                    for rt in range(RT):
                        nc.tensor.transpose(
                            pt[:, rt * 128 : (rt + 1) * 128],
                            o16[rt][:, hc * 128 : (hc + 1) * 128],
                            ident[:, :],
                        )
                    nc.vector.tensor_scalar(
                        no_8[:, hc, :], pt[:, :], FS, None, ALU.mult
                    )

                # output-only work, off the recurrence critical path: emit
                # after the o8 chain so it queues behind it on the DVE
                for rt in range(RT):
                    o32 = smx.tile([128, H], f32, tag=f"o32_{rt}")
                    nc.vector.tensor_copy(o32[:, :], o16[rt][:, :])
                    nc.sync.dma_start(
                        out=out_v[rt * 4 : (rt + 1) * 4, t, :, :], in_=o32[:, :]
                    )

                cur_h0, cur_h1, cur_o = nh0_8, nh1_8, no_8

    return nc


def _get_nc():
    if "nc" not in _cache:
        nc = _build_nc()
        if not nc.is_finalized():
            nc.finalize()
        _cache["nc"] = nc
    return _cache["nc"]


def _f8(x):
    import ml_dtypes

    return np.clip(x, -240.0, 240.0).astype(ml_dtypes.float8_e4m3)


def _pack8(mat, scale):
    """[H, F] (k-major) -> [128, KT, F] fp8, scaled."""
    h, f = mat.shape
    assert h == H
    return _f8(np.ascontiguousarray(mat.reshape(KT, 128, f).transpose(1, 0, 2)) * scale)


def _make_in_maps(inputs):
    x = {k: np.asarray(v) for k, v in inputs.items()}
    W_ih0 = x["W_ih0"].astype(np.float32)
    W_hh0 = x["W_hh0"].astype(np.float32)
    W_ih1 = x["W_ih1"].astype(np.float32)
    W_hh1 = x["W_hh1"].astype(np.float32)
    W_out = x["W_out"].astype(np.float32)
    b0 = (x["b_ih0"] + x["b_hh0"]).astype(np.float32)
    b1 = (x["b_ih1"] + x["b_hh1"]).astype(np.float32)
    b_out = x["b_out"].astype(np.float32)
    emb = x["embedding_C"].astype(np.float32)
    h0 = x["h0"].astype(np.float32)
    c0 = x["c0"].astype(np.float32)
    out0 = x["out0"].astype(np.float32)

    FS2_np = np.float32(FS * FS)
    shared = {
        "w0a": _pack8(np.ascontiguousarray(W_ih0[:, :H].T), FS),
        "w0b": np.ascontiguousarray(W_ih0[:, H:].T).astype(np.float16),
        "w0h": _pack8(np.ascontiguousarray(W_hh0.T), FS),
        "w1i": _pack8(np.ascontiguousarray(W_ih1.T), FS),
        "w1h": _pack8(np.ascontiguousarray(W_hh1.T), FS),
        "wo": np.ascontiguousarray(W_out.T).astype(np.float16),
        "b0": np.ascontiguousarray((b0 * FS2_np).reshape(GT, 128).T),
        "b1h": np.ascontiguousarray((b1 * 0.5).reshape(GT, 128).T),
        "b1f": np.ascontiguousarray(b1.reshape(GT, 128).T),
        "bo_row": np.ascontiguousarray((b_out * FS).reshape(1, H)).astype(np.float16),
    }

    def t16(rows_by_h):  # [R, H] -> [H, R] fp16
        return np.ascontiguousarray(rows_by_h.T).astype(np.float16)

    in_maps = []
    for c in range(NCORES):
        bs = slice(c * BPC, (c + 1) * BPC)
        m = dict(shared)
        m["embT"] = t16(np.swapaxes(emb[bs], 0, 1).reshape(R, H))
        m["h0T8"] = _pack8(h0[:, 0, bs, :].reshape(R, H).T, FS)
        m["h1T8"] = _pack8(h0[:, 1, bs, :].reshape(R, H).T, FS)
        m["c0T"] = t16(c0[:, 0, bs, :].reshape(R, H))
        m["c1T"] = t16(c0[:, 1, bs, :].reshape(R, H))
        m["oT8"] = _pack8(out0[:, bs, :].reshape(R, H).T, FS)
        in_maps.append(m)
    return in_maps


def kernel(**inputs):
    global LAST_EXEC_NS
    _ensure_path()
    from concourse.bass_utils import run_bass_kernel_spmd

    in_maps = _make_in_maps(inputs)
    nc = _get_nc()
    trace = bool(os.environ.get("KTRACE"))
    kw = {}
    if trace and os.environ.get("KTRACE_DIR"):
        os.makedirs(os.environ["KTRACE_DIR"], exist_ok=True)
        kw["tmpdir"] = os.environ["KTRACE_DIR"]
    try:
        res = run_bass_kernel_spmd(nc, in_maps, list(range(NCORES)), trace=trace, **kw)
    except (ImportError, ModuleNotFoundError):
        res = run_bass_kernel_spmd(nc, in_maps, list(range(NCORES)), trace=False)
    if getattr(res, "exec_time_ns", None):
        LAST_EXEC_NS = res.exec_time_ns

    outs = [np.asarray(res.results[c]["out"], dtype=np.float32) for c in range(NCORES)]
    return np.concatenate(outs, axis=0)


if __name__ == "__main__":
    nc = _get_nc()
    print("built ok")
